# revision 37
# baseline (speedup 1.0000x reference)
"""DiffAttention Trainium2 kernel, 8-core SPMD (head-parallel), fp16 attention.

Problem (hardcoded): B=2, S=2048, D=128, H=8.
  q = (x@Wq.T+bq).reshape(B,H,S,2D)   # raw reshape: head h <-> rows [256h,256h+256) of proj
  s1 = q1@k1.T; s2 = q2@k2.T; attn = softmax(s1) - lam*softmax(s2)
  out = attn@v -> transpose/reshape -> GroupNorm(H groups) -> *(1-lam) -> concat heads -> @Wo.T+bo

Sharding: core c owns head h=c for both batches (2 units/core). GroupNorm groups
mix all heads -> tiny (16-float) AllGather of partial stats per unit.

Numerics: q/k/v/exp in fp16 (PSUM accumulation f32). Softmax computed without
max-subtraction but with a constant shift (softmax-invariant) to keep exp in
fp16/fp8 range. The softmax denominator R is computed with an fp8e5 DoubleRow
matmul reading the HIGH BYTE of the fp16 exp tile (fp16 truncates to fp8e5m2
bytewise); the resulting systematic ~-9% scale on R cancels exactly in the
GroupNorm normalization. A subset of exp tiles is computed on DVE with a
Schraudolph-style bit-trick (linear map into fp16 bits) to offload the ACT
engine; its per-element error (~1.5%) washes out over the 2048-key softmax
sums. Final projection folds the GroupNorm scale A into Wo and the constant
cb into a rank-1 PSUM-accumulated matmul.

Index algebra per unit (b,h), block = proj rows [256h, 256h+256):
  sigma (attn row) = 8r+j, r in [0,256), j in [0,8). We use tau-order sigma' = 256j+r.
  q1T[d, sigma'=256j+r] = qpT_block[f=256j+d, r]   (even 128-col chunks of qp block)
  q2T: odd chunks.  v'[sigma'=256j+r, d] = vp_block[r, 128j+d].
  GroupNorm group g = {sigma': (sigma' mod 256)//32 == g} (32-wide strips).
  OT layout per unit: [128 d, 2048] cols = (g8, qb4, j'2, r32) so the
  final Wo-matmul lhsT blocks are contiguous (walrus: weights AP must have
  a single free dimension).
  Final rows: device row (rh, m'') holds rho = 8*(m'' % 32) + 4*rh + m''//32.
"""

import sys

sys.path.insert(0, "/opt/trn_rl_repo")

import numpy as np

import concourse.bacc as bacc
import concourse.bass_isa as bass_isa
import concourse.mybir as mybir
import concourse.tile as tile

F32 = mybir.dt.float32
F32R = mybir.dt.float32r
F16 = mybir.dt.float16
F8E5 = mybir.dt.float8e5
AF = mybir.ActivationFunctionType
ALU = mybir.AluOpType
PM = mybir.MatmulPerfMode

B, S, D, H = 2, 2048, 128, 8
N_CORES = 8
EPS = 1e-5
SHIFT = 3.0  # exp(s - SHIFT): softmax-shift-invariant, keeps exp in fp16 range
DVE_EXP = True
LOG2E = 1.4426950408889634
A_EXP16 = 1024.0 * LOG2E              # Schraudolph fp16-bit exp on DVE
C_EXP16 = 1024.0 * 15 - 3.0 * A_EXP16 - 66.0  # folds SHIFT; tuned for trunc
GROUP_N = float(256 * H * D)  # elements per GroupNorm group

_CACHED = None


def build_nc():
    nc = bacc.Bacc("TRN2", target_bir_lowering=False, debug=False, num_devices=N_CORES)

    # ---- per-core external I/O ----
    qT = nc.dram_tensor("qT", [B, 128, 256], F32, kind="ExternalInput")  # query block.T per batch
    wqT = nc.dram_tensor("wqT", [128, 2048], F32, kind="ExternalInput")
    wkT = nc.dram_tensor("wkT", [128, 2048], F32, kind="ExternalInput")
    wvT = nc.dram_tensor("wvT", [128, 1024], F32, kind="ExternalInput")
    woT = nc.dram_tensor("woT", [1024, 128], F32, kind="ExternalInput")
    bqT = nc.dram_tensor("bqT", [128, 16], F32, kind="ExternalInput")
    bkT = nc.dram_tensor("bkT", [128, 16], F32, kind="ExternalInput")
    bv = nc.dram_tensor("bv", [1, 1024], F32, kind="ExternalInput")
    bo = nc.dram_tensor("bo", [1, 128], F32, kind="ExternalInput")
    gnw2 = nc.dram_tensor("gnw2", [1, 16], F32, kind="ExternalInput")  # tiled x2 (b,g)
    gnb2 = nc.dram_tensor("gnb2", [1, 16], F32, kind="ExternalInput")
    lam = nc.dram_tensor("lam", [1, 1], F32, kind="ExternalInput")
    wsum = nc.dram_tensor("wsum", [1, 1024], F32, kind="ExternalInput")
    outp = nc.dram_tensor("outp", [B, 256, 128], F32, kind="ExternalOutput")

    with tile.TileContext(nc) as tc:
        with (
            tc.tile_pool(name="const", bufs=1) as cpool,
            tc.tile_pool(name="proj", bufs=2) as projpool,
            tc.tile_pool(name="vpool", bufs=4) as vpool,
            tc.tile_pool(name="epool", bufs=5) as epool,
            tc.tile_pool(name="tmp", bufs=2) as tmppool,
            tc.tile_pool(name="ps_s", bufs=2, space="PSUM") as ps_s,
            tc.tile_pool(name="ps_acc", bufs=1, space="PSUM") as ps_acc,
            tc.tile_pool(name="dram", bufs=1, space="DRAM") as dram,
        ):
            # ---- load constants / weights (qT first: projections need it) ----
            qt_sb = []
            qt_r = []
            for u in range(B):
                q = cpool.tile([128, 256], F32, name=f"qt_sb{u}")
                nc.sync.dma_start(q[:], qT[u])
                qt_sb.append(q)
                qr = cpool.tile([128, 256], F32R, name=f"qt_r{u}")
                nc.vector.tensor_copy(qr[:], q[:])
                qt_r.append(qr)

            # small constants go on the gpsimd DMA queue so they don't delay
            # the big weight DMAs on the sync queue
            bq_sb = cpool.tile([128, 16], F32)
            bk_sb = cpool.tile([128, 16], F32)
            nc.gpsimd.dma_start(bq_sb[:], bqT[:])
            nc.gpsimd.dma_start(bk_sb[:], bkT[:])
            bv_sb = cpool.tile([1, 1024], F32)
            nc.gpsimd.dma_start(bv_sb[:], bv[:])
            bo_sb = cpool.tile([1, 128], F32)
            nc.gpsimd.dma_start(bo_sb[:], bo[:])
            gnw_sb = cpool.tile([1, 16], F32)
            gnb_sb = cpool.tile([1, 16], F32)
            nc.gpsimd.dma_start(gnw_sb[:], gnw2[:])
            nc.gpsimd.dma_start(gnb_sb[:], gnb2[:])
            lam_sb = cpool.tile([1, 1], F32)
            nc.gpsimd.dma_start(lam_sb[:], lam[:])

            shift_sb = cpool.tile([128, 1], F32)
            nc.vector.memset(shift_sb[:], -SHIFT)
            ones5 = cpool.tile([128, 256], F8E5)
            nc.vector.memset(ones5[:], 1.0)
            ones_c16 = cpool.tile([1, 128], F16)
            nc.vector.memset(ones_c16[:], 1.0)
            # preload the Exp activation table while DMAs run
            scr1 = cpool.tile([128, 1], F32)
            nc.scalar.activation(scr1[:], shift_sb[:], AF.Exp, bias=shift_sb[:])

            # weights loaded and rounded in 1024-col pieces so projections
            # (and then attention) can start before all input DMA completes.
            wq_rh, wk_rh = [], []
            wv_r = cpool.tile([128, 1024], F32R)
            # 512-col pieces, ordered by first use in the attention stream
            wq_rh = [cpool.tile([128, 1024], F32R, name=f"wq_r{h}") for h in range(2)]
            wk_rh = [cpool.tile([128, 1024], F32R, name=f"wk_r{h}") for h in range(2)]
            wpieces = [
                ("wq", wqT, wq_rh[0], 0), ("wk", wkT, wk_rh[0], 0),
                ("wq", wqT, wq_rh[0], 1), ("wk", wkT, wk_rh[0], 1),
                ("wv", wvT, wv_r, 0), ("wv", wvT, wv_r, 1),
                ("wk", wkT, wk_rh[1], 2), ("wk", wkT, wk_rh[1], 3),
                ("wq", wqT, wq_rh[1], 2), ("wq", wqT, wq_rh[1], 3),
            ]
            for (wnm, dram_w, dst_t, piece) in wpieces:
                wsc = projpool.tile([128, 512], F32, tag="wsc", name=f"wsc_{wnm}{piece}", bufs=3)
                nc.sync.dma_start(wsc[:], dram_w[:, 512 * piece : 512 * (piece + 1)])
                col = 512 * (piece % 2)
                nc.vector.tensor_copy(dst_t[:, col : col + 512], wsc[:])
            lam_rep = cpool.tile([128, 1], F32)
            nc.gpsimd.partition_broadcast(lam_rep[:], lam_sb[:])
            oml = cpool.tile([1, 1], F32)
            nc.vector.tensor_scalar(oml[:], lam_sb[:], -1.0, 1.0, ALU.mult, ALU.add)
            bv_rep = cpool.tile([128, 1024], F32)
            nc.gpsimd.partition_broadcast(bv_rep[:], bv_sb[:])
            # Wo -> fp16 (single staged tile); column sums come precomputed
            # from the host (wsum input). DMAs ride the gpsimd queue; the fp16
            # convert is emitted after unit-0 attention so it doesn't block
            # the startup DVE queue.
            wo_st = cpool.tile([128, 1024], F32)
            for h3 in range(8):
                nc.gpsimd.dma_start(wo_st[:, 128 * h3 : 128 * (h3 + 1)],
                                    woT[128 * h3 : 128 * (h3 + 1), :])
            wsum_sb = cpool.tile([1, 1024], F32)
            nc.gpsimd.dma_start(wsum_sb[:], wsum[:])
            wo_16 = cpool.tile([128, 1024], F16)

            # ================= projections (both units), fp16 outputs =========
            p2_tiles = {0: [], 1: []}
            OT = []  # [128, 2048] fp16 per unit: cols (qb4, j'2, g8, r32)
            for u in range(B):
                ot = cpool.tile([128, 2048], F16, name=f"OT_{u}")
                OT.append(ot)
            proj = []  # (q1t, q2t, k1t, k2t, vp) per unit
            qk = {}
            for u in range(B):
                for nm in ("q1", "q2"):
                    qk[(u, nm)] = [
                        projpool.tile([128, 512], F16, tag=f"{nm}t",
                                      name=f"{nm}t_{u}_{qb}", bufs=8)
                        for qb in range(4)
                    ]
                for nm in ("k1", "k2"):
                    qk[(u, nm)] = [
                        projpool.tile([128, 1024], F16, tag=f"{nm}t",
                                      name=f"{nm}t_{u}_{hh}", bufs=4)
                        for hh in range(2)
                    ]
            vps = [None, None]
            vtiles = {}

            def proj_thunks(u, part, adds_on_act):
                # bias-adds ride ACT (idle in the startup phase) or DVE
                def badd(dst, ps, b):
                    if adds_on_act:
                        nc.scalar.add(dst, ps, b)
                    else:
                        nc.vector.tensor_scalar_add(dst, ps, b)

                def qproj(j):
                    ps = ps_s.tile([128, 256], F32, tag="s", name=f"pp_{u}_{j}")
                    nc.tensor.matmul(
                        ps[:], wq_rh[j // 8][:, 128 * (j % 8) : 128 * (j % 8 + 1)],
                        qt_r[u][:], start=True, stop=True,
                    )
                    dst = qk[(u, "q1" if j % 2 == 0 else "q2")][j // 4]
                    col = 256 * ((j // 2) % 2)
                    badd(dst[:, col : col + 256], ps[:], bq_sb[:, j : j + 1])

                def kproj(j):
                    ps = ps_s.tile([128, 256], F32, tag="s", name=f"pk_{u}_{j}")
                    nc.tensor.matmul(
                        ps[:], wk_rh[j // 8][:, 128 * (j % 8) : 128 * (j % 8 + 1)],
                        qt_r[u][:], start=True, stop=True,
                    )
                    dst = qk[(u, "k1" if j % 2 == 0 else "k2")][j // 8]
                    col = 256 * ((j // 2) % 4)
                    badd(dst[:, col : col + 256], ps[:], bk_sb[:, j : j + 1])

                def vpiece(rc, fh):
                    vt = vtiles[(u, rc)]
                    ps = ps_s.tile([128, 512], F32, tag="s", name=f"ppv_{u}_{rc}_{fh}")
                    nc.tensor.matmul(
                        ps[:], qt_r[u][:, 128 * rc : 128 * (rc + 1)],
                        wv_r[:, 512 * fh : 512 * (fh + 1)],
                        start=True, stop=True,
                    )
                    nc.vector.tensor_tensor(
                        vt[:, 512 * fh : 512 * (fh + 1)], ps[:],
                        bv_rep[:, 512 * fh : 512 * (fh + 1)], ALU.add,
                    )

                # attention qb0 needs q j0..3 and k j0..7 (wq0/wk0 land first)
                if part == "qk_lo":
                    return ([lambda j=j: qproj(j) for j in range(8)] +
                            [lambda j=j: kproj(j) for j in range(8)])
                elif part == "v":
                    if (u, 0) not in vtiles:
                        for rc in range(2):
                            vtiles[(u, rc)] = vpool.tile(
                                [128, 1024], F16, tag="vp", name=f"vp_{u}_{rc}")
                        vps[u] = [vtiles[(u, 0)], vtiles[(u, 1)]]
                    return [lambda rc=rc, fh=fh: vpiece(rc, fh)
                            for fh in range(2) for rc in range(2)]
                else:  # qk_hi: k first (needed at qb0 kcg4), q later (qb2)
                    return ([lambda j=j: kproj(j) for j in range(8, 16)] +
                            [lambda j=j: qproj(j) for j in range(8, 16)])

            def emit_proj(u, part, adds_on_act):
                for th in proj_thunks(u, part, adds_on_act):
                    th()

            for u in range(B):
                proj.append((qk[(u, "q1")], qk[(u, "q2")],
                             qk[(u, "k1")], qk[(u, "k2")], None))

            cc_in = [dram.tile([1, 16], F32, name=f"cc_in{u}") for u in range(B)]
            cc_out = [dram.tile([1, 128], F32, addr_space="Shared", name=f"cc_out{u}")
                      for u in range(B)]
            gath = [tmppool.tile([1, 128], F32, tag="gath", name=f"gath_{u}", bufs=2)
                    for u in range(B)]
            scal = [{}, {}]
            wo_scA = [cpool.tile([128, 1024], F16, name=f"wo_scA{u}") for u in range(B)]

            def emit_stats_export(u):
                # fold (j mod 2) pairs -> per-group partials, reduce partitions,
                # then AllGather the 16 floats across cores.
                stats_u = tmppool.tile([1, 16], F32, tag="stats", name=f"stats_{u}", bufs=2)
                for si, p1x in enumerate(p2_tiles[u]):
                    p2 = tmppool.tile([128, 8], F32, tag="p2", name=f"p2_{u}_{si}", bufs=4)
                    nc.vector.tensor_reduce(
                        p2[:], p1x[:].rearrange("p (g j) -> p g j", g=8, j=2),
                        mybir.AxisListType.X, ALU.add,
                    )
                    pr = tmppool.tile([128, 8], F32, tag="pr", name=f"pr_{u}_{si}", bufs=4)
                    nc.gpsimd.partition_all_reduce(pr[:], p2[:], 128, bass_isa.ReduceOp.add)
                    nc.vector.tensor_copy(stats_u[:, 8 * si : 8 * si + 8], pr[0:1, :])
                nc.gpsimd.dma_start(cc_in[u][:], stats_u[:])
                nc.gpsimd.collective_compute(
                    "AllGather", ALU.bypass,
                    replica_groups=[list(range(N_CORES))],
                    ins=[cc_in[u][:]], outs=[cc_out[u][:]],
                )
                nc.gpsimd.dma_start(gath[u][:], cc_out[u][:])

            def emit_scalars(u):
                # global stats for batch u -> A_rep[128,8], cb_rep[128,128],
                # and A-folded Wo (wo_scA[u]).
                t = lambda nm: tmppool.tile([1, 8], F32, tag=nm, name=f"{nm}_{u}", bufs=2)
                # gathered stats live in a [1,128] row: one strided reduce
                # sums the 8 per-core chunks (cheaper than a gpsimd round-trip)
                g1 = tmppool.tile([1, 16], F32, tag="gr", name=f"gr_{u}", bufs=2)
                nc.vector.tensor_reduce(
                    g1[:], gath[u][:].rearrange("p (c s) -> p s c", c=8, s=16),
                    mybir.AxisListType.X, ALU.add,
                )
                glob = g1[0:1, :]
                mean, ex2, var, veps = t("mean"), t("ex2"), t("var"), t("veps")
                nc.vector.tensor_scalar_mul(mean[:], glob[:, 0:8], 1.0 / GROUP_N)
                nc.vector.tensor_scalar_mul(ex2[:], glob[:, 8:16], 1.0 / GROUP_N)
                nc.vector.tensor_tensor(var[:], mean[:], mean[:], ALU.mult)
                nc.vector.tensor_tensor(var[:], ex2[:], var[:], ALU.subtract)
                nc.vector.tensor_scalar_add(veps[:], var[:], EPS)
                # rsqrt fully on DVE (ACT Sqrt would thrash the exp table set):
                # quake seed + 2 Newton steps
                I32 = mybir.dt.int32
                ti = tmppool.tile([1, 8], I32, tag="rsqi", name=f"rsqi_{u}", bufs=2)
                nc.vector.tensor_scalar(
                    ti[:], veps[:].bitcast(I32), 1, None, ALU.arith_shift_right
                )
                nc.vector.tensor_scalar(ti[:], ti[:], -1, 0x5F3759DF, ALU.mult, ALU.add)
                rstd, hf, nt = t("rstd"), t("hf"), t("nt")
                nc.vector.tensor_copy(rstd[:], ti[:].bitcast(F32))
                nc.vector.tensor_scalar_mul(hf[:], veps[:], 0.5)
                for _ in range(2):
                    nc.vector.tensor_tensor(nt[:], rstd[:], rstd[:], ALU.mult)
                    nc.vector.tensor_tensor(nt[:], nt[:], hf[:], ALU.mult)
                    nc.vector.tensor_scalar(nt[:], nt[:], -1.0, 1.5, ALU.mult, ALU.add)
                    nc.vector.tensor_tensor(rstd[:], rstd[:], nt[:], ALU.mult)
                A, Bc = t("A"), t("Bc")
                nc.vector.tensor_tensor(A[:], rstd[:], gnw_sb[:, 0:8], ALU.mult)
                nc.vector.tensor_tensor(Bc[:], mean[:], A[:], ALU.mult)
                nc.vector.tensor_tensor(Bc[:], gnb_sb[:, 0:8], Bc[:], ALU.subtract)
                nc.vector.tensor_scalar_mul(A[:], A[:], oml[:, 0:1])
                nc.vector.tensor_scalar_mul(Bc[:], Bc[:], oml[:, 0:1])
                # A broadcast via a rank-1 PE matmul (no gpsimd in the tail)
                A16 = tmppool.tile([1, 8], F16, tag="A16", name=f"A16_{u}", bufs=2)
                nc.vector.tensor_copy(A16[:], A[:])
                aps = ps_s.tile([128, 8], F32, tag="s", name=f"aps_{u}")
                nc.tensor.matmul(aps[:], ones_c16[:], A16[:], start=True, stop=True)
                A_rep = tmppool.tile([128, 8], F32, tag="A_rep", name=f"A_rep{u}", bufs=2)
                nc.vector.tensor_copy(A_rep[:], aps[:])
                cb = tmppool.tile([1, 128], F32, tag="cb", name=f"cb_{u}", bufs=2)
                nc.vector.tensor_scalar_mul(cb[:], wsum_sb[:, 0:128], Bc[:, 0:1])
                for h3 in range(1, 8):
                    nc.vector.scalar_tensor_tensor(
                        cb[:], wsum_sb[:, 128 * h3 : 128 * (h3 + 1)],
                        Bc[:, h3 : h3 + 1], cb[:], ALU.mult, ALU.add,
                    )
                nc.vector.tensor_tensor(cb[:], cb[:], bo_sb[:], ALU.add)
                cb16 = tmppool.tile([1, 128], F16, tag="cb16", name=f"cb16_{u}", bufs=2)
                nc.vector.tensor_copy(cb16[:], cb[:])
                scal[u] = {"A_rep": A_rep, "cb16": cb16}
                for h3 in range(8):
                    nc.vector.tensor_scalar_mul(
                        wo_scA[u][:, 128 * h3 : 128 * (h3 + 1)],
                        wo_16[:, 128 * h3 : 128 * (h3 + 1)],
                        A_rep[:, h3 : h3 + 1],
                    )

            # ================= attention =================
            # The consume pipeline (pending) carries across q-block and unit
            # boundaries so the PE/ACT streams never drain; each block's
            # O-combine is emitted when its last consume pops.
            o5v = ones5[:].rearrange("p (two m) -> p two m", two=2)
            pending = []

            def consume(item):
                (u, qb, kcg, eg, uacc, racc, vp, is_last) = item
                for h in range(2):
                    kc = 2 * kcg + h
                    vchunk = vp[kc % 2][:, 128 * (kc // 2) : 128 * (kc // 2) + 128]
                    nc.tensor.matmul(
                        uacc[:], vchunk, eg[:, 512 * h : 512 * (h + 1)],
                        start=(kcg == 0 and h == 0),
                        stop=(kcg == 7 and h == 1),
                    )
                # denominator via fp8e5 DoubleRow on the hi-byte view
                e8v = (eg[:].bitcast(F8E5)
                       .rearrange("p (two n b) -> p two n b", two=2, n=512, b=2)
                       [:, :, :, 1:2])
                nc.tensor.matmul(
                    racc[:], o5v, e8v,
                    start=(kcg == 0), stop=(kcg == 7),
                    perf_mode=PM.DoubleRow,
                )
                if is_last:
                    phase, args = is_last
                    if phase == 0:
                        ocombine_m0(*args)
                    else:
                        ocombine_m1(*args)

            def pop_pending():
                if len(pending) > 4:
                    consume(pending.pop(0))

            def flush_pending():
                while pending:
                    consume(pending.pop(0))

            oc_t1 = {}

            def ocombine_m0(u, qb, u1, u2, r1, r2, p1a, p1b):
                # m0-half right after the m0 accumulators close: frees u1/r1
                # a full iteration earlier than the m1-half
                r1i = tmppool.tile([128, 512], F32, tag="r1i", name=f"r1i_{u}_{qb}")
                t1 = tmppool.tile([128, 512], F32, tag="t1", name=f"t1_{u}_{qb}")
                nc.vector.reciprocal(r1i[:], r1[:])
                nc.vector.tensor_tensor(t1[:], u1[:], r1i[:], ALU.mult)
                oc_t1[(u, qb)] = t1

            def ocombine_m1(u, qb, u1, u2, r1, r2, p1a, p1b):
                # O = U1/R1 - lam*U2/R2   (R replicated across partitions)
                t1 = oc_t1.pop((u, qb))
                r2i = tmppool.tile([128, 512], F32, tag="r2i", name=f"r2i_{u}_{qb}")
                t2 = tmppool.tile([128, 512], F32, tag="t2", name=f"t2_{u}_{qb}")
                nc.vector.reciprocal(r2i[:], r2[:])
                nc.vector.scalar_tensor_tensor(
                    t2[:], u2[:], lam_rep[:, 0:1], r2i[:], ALU.mult, ALU.mult
                )
                # OT layout is (g, qb, j', r); src t1/t2 cols are
                # (j', g, r) -> write through matching strided views
                ot5 = OT[u][:].rearrange("p (g qb j r) -> p g qb j r",
                                         g=8, qb=4, j=2, r=32)
                osl = ot5[:, :, qb, :, :]
                tview = lambda t: t[:].rearrange("p (j g r) -> p g j r",
                                                 j=2, g=8, r=32)
                nc.vector.tensor_tensor(osl, tview(t1), tview(t2), ALU.subtract)

                # incremental GroupNorm partial stats for this q-block;
                # strip sums come out in (g, j') column order
                red = tmppool.tile([128, 16], F32, tag="red", name=f"red_{u}_{qb}")
                nc.vector.tensor_reduce(red[:], osl, mybir.AxisListType.X, ALU.add)
                if qb == 0:
                    nc.vector.tensor_copy(p1a[:], red[:])
                else:
                    nc.vector.tensor_tensor(p1a[:], p1a[:], red[:], ALU.add)
                sq5 = tmppool.tile([128, 512], F32, tag="t1", name=f"sq5_{u}_{qb}")
                nc.scalar.square(sq5[:].rearrange("p (g j r) -> p g j r", g=8, j=2, r=32), osl)
                redb = tmppool.tile([128, 16], F32, tag="redb", name=f"redb_{u}_{qb}")
                nc.vector.tensor_reduce(
                    redb[:], sq5[:].rearrange("p (g j r) -> p g j r", g=8, j=2, r=32),
                    mybir.AxisListType.X, ALU.add,
                )
                if qb == 0:
                    nc.vector.tensor_copy(p1b[:], redb[:])
                else:
                    nc.vector.tensor_tensor(p1b[:], p1b[:], redb[:], ALU.add)
                if qb == 3:
                    p2_tiles[u] = [p1a, p1b]
                    emit_stats_export(u)

            def emit_attention(u, work=()):
                q1l, q2l, k1l, k2l, _ = proj[u]
                vp = vps[u]
                p1a = tmppool.tile([128, 16], F32, tag="p1a", name=f"p1a_{u}")
                p1b = tmppool.tile([128, 16], F32, tag="p1b", name=f"p1b_{u}")

                for qb in range(4):
                    u1 = ps_acc.tile([128, 512], F32, tag="u1", name=f"u1_{u}_{qb}")
                    u2 = ps_acc.tile([128, 512], F32, tag="u2", name=f"u2_{u}_{qb}")
                    r1 = ps_acc.tile([128, 512], F32, tag="r1", name=f"r1_{u}_{qb}")
                    r2 = ps_acc.tile([128, 512], F32, tag="r2", name=f"r2_{u}_{qb}")
                    for kcg in range(8):
                        for m, (kl, qtile, uacc, racc) in enumerate(
                            ((k1l, q1l[qb], u1, r1), (k2l, q2l[qb], u2, r2))
                        ):
                            if work:
                                work.pop(0)()
                            sgrp = ps_s.tile([128, 1024], F32, tag="s", name=f"s_{u}_{qb}_{kcg}_{m}")
                            for h in range(2):
                                kc = 2 * kcg + h
                                nc.tensor.matmul(
                                    sgrp[:, 512 * h : 512 * (h + 1)],
                                    kl[kc // 8][:, 128 * (kc % 8) : 128 * (kc % 8 + 1)],
                                    qtile[:],
                                    start=True, stop=True,
                                )
                            eg = epool.tile([128, 1024], F16, tag="e", name=f"e_{u}_{qb}_{kcg}_{m}")
                            if DVE_EXP and ((u == 1 and kcg % 4 == (2 * qb + m) % 4) or (u == 0 and qb == 3 and kcg == 4 + m)):
                                # bit-trick exp on DVE to offload the ACT engine
                                nc.vector.tensor_scalar(
                                    eg[:].bitcast(mybir.dt.int16), sgrp[:],
                                    A_EXP16, C_EXP16, ALU.mult, ALU.add,
                                )
                            else:
                                nc.scalar.activation(eg[:], sgrp[:], AF.Exp, bias=shift_sb[:])
                            is_last = ((m, (u, qb, u1, u2, r1, r2, p1a, p1b))
                                       if kcg == 7 else None)
                            pending.append((u, qb, kcg, eg, uacc, racc, vp, is_last))
                            pop_pending()

            def emit_output(u):
                # out rows = A-folded P matmuls + rank-1 cb fold, all in PSUM
                cb16 = scal[u]["cb16"]
                for rh in range(2):
                    pout = ps_s.tile([128, 128], F32, tag="s", name=f"po_{u}_{rh}")
                    for h3 in range(8):
                        base = 256 * h3 + 128 * rh
                        lhsT = OT[u][:, base : base + 128]
                        nc.tensor.matmul(
                            pout[:], lhsT,
                            wo_scA[u][:, 128 * h3 : 128 * (h3 + 1)],
                            start=(h3 == 0), stop=False,
                        )
                    nc.tensor.matmul(pout[:], ones_c16[:], cb16[:],
                                     start=False, stop=True)
                    rsb = tmppool.tile([128, 128], F32, tag="rsb", name=f"rsb_{u}_{rh}")
                    nc.vector.tensor_copy(rsb[:], pout[:])
                    # contiguous block write; host undoes the row permutation
                    # (device row 128*rh+m'' holds rho = 8*(m''%32)+4*rh+m''//32)
                    nc.sync.dma_start(outp[u][128 * rh : 128 * (rh + 1), :], rsb[:])

            emit_proj(0, "qk_lo", adds_on_act=True)
            noop = lambda: None
            vth0 = proj_thunks(0, "v", adds_on_act=False)
            work0 = (
                vth0 +                      # v pieces just-in-time for consume
                [noop] +
                proj_thunks(0, "qk_hi", adds_on_act=False) +
                proj_thunks(1, "qk_lo", adds_on_act=False) +
                proj_thunks(1, "v", adds_on_act=False) +
                proj_thunks(1, "qk_hi", adds_on_act=False)
            )
            emit_attention(0, work=work0)
            nc.vector.tensor_copy(wo_16[:], wo_st[:])
            # unit-0's last O-combine + stats export (collective #0) pop
            # inside unit-1's stream; collective #0 overlaps u1 attention
            emit_attention(1, work=work0)
            flush_pending()             # u1 qb3 O-combine + stats export #1
            emit_scalars(0)             # collective #0 landed long ago
            emit_output(0)              # overlaps collective #1
            emit_scalars(1)             # waits on collective #1
            emit_output(1)

    nc.compile()
    return nc


def _prep_inputs(inputs):
    """Host-side: slice/transpose full inputs into per-core in_maps."""
    query = np.asarray(inputs["query"], np.float32)
    Wq = np.asarray(inputs["Wq"], np.float32)
    Wk = np.asarray(inputs["Wk"], np.float32)
    Wv = np.asarray(inputs["Wv"], np.float32)
    Wo = np.asarray(inputs["Wo"], np.float32)
    bq = np.asarray(inputs["bq"], np.float32)
    bk = np.asarray(inputs["bk"], np.float32)
    bv = np.asarray(inputs["bv"], np.float32)
    bo = np.asarray(inputs["bo"], np.float32)
    gn_w = np.asarray(inputs["gn_w"], np.float32)
    gn_b = np.asarray(inputs["gn_b"], np.float32)
    lam = np.asarray(inputs["lam"], np.float32).reshape(1, 1)

    shared = {
        "wqT": np.ascontiguousarray(Wq.T),
        "wkT": np.ascontiguousarray(Wk.T),
        "wvT": np.ascontiguousarray(Wv.T),
        "woT": np.ascontiguousarray(Wo.T),
        "bqT": np.ascontiguousarray(bq.reshape(16, 128).T),
        "bkT": np.ascontiguousarray(bk.reshape(16, 128).T),
        "bv": bv.reshape(1, 1024),
        "bo": bo.reshape(1, 128),
        "gnw2": np.tile(gn_w, 2).reshape(1, 16),
        "gnb2": np.tile(gn_b, 2).reshape(1, 16),
        "lam": lam,
        "wsum": np.ascontiguousarray(
            Wo.reshape(128, 8, 128).sum(-1).T.reshape(1, 1024).astype(np.float32)),
    }
    in_maps = []
    for c in range(N_CORES):
        blk = query[:, 256 * c : 256 * (c + 1), :]  # [B, 256, 128]
        qT = np.ascontiguousarray(blk.transpose(0, 2, 1))  # [B, 128, 256]
        in_maps.append({"qT": qT, **shared})
    return in_maps


class _Runner:
    """Cached-jit SPMD executor (one trace/compile; cheap repeated calls)."""

    def __init__(self, nc):
        import jax
        from jax.sharding import Mesh, PartitionSpec
        from jax.experimental.shard_map import shard_map
        from concourse.bass2jax import (
            install_neuronx_cc_hook, _bass_exec_p, partition_id_tensor,
        )

        install_neuronx_cc_hook()
        self.jax = jax
        pname = nc.partition_id_tensor.name if nc.partition_id_tensor else None
        in_names, out_names, out_avals, zero_outs = [], [], [], []
        for alloc in nc.m.functions[0].allocations:
            if not isinstance(alloc, mybir.MemoryLocationSet):
                continue
            name = alloc.memorylocations[0].name
            if alloc.kind == "ExternalInput":
                if name != pname:
                    in_names.append(name)
            elif alloc.kind == "ExternalOutput":
                out_names.append(name)
                shape = tuple(alloc.tensor_shape)
                dtype = mybir.dt.np(alloc.dtype)
                out_avals.append(jax.core.ShapedArray(shape, dtype))
                zero_outs.append(np.zeros(shape, dtype))
        self.in_names, self.out_names = in_names, out_names
        n_params = len(in_names)
        all_names = list(in_names) + out_names
        if pname is not None:
            all_names.append(pname)

        def _body(*args):
            operands = list(args)
            if pname is not None:
                operands.append(partition_id_tensor())
            return tuple(_bass_exec_p.bind(
                *operands, out_avals=tuple(out_avals), in_names=tuple(all_names),
                out_names=tuple(out_names), lowering_input_output_aliases=(),
                sim_require_finite=True, sim_require_nnan=True, nc=nc))

        devices = jax.devices()[:N_CORES]
        mesh = Mesh(np.asarray(devices), ("core",))
        nio = n_params + len(out_names)
        self.fn = jax.jit(
            shard_map(_body, mesh=mesh, in_specs=(PartitionSpec("core"),) * nio,
                      out_specs=(PartitionSpec("core"),) * len(out_names),
                      check_rep=False),
            keep_unused=True,
        )
        self.zeros = [
            jax.device_put(np.zeros((N_CORES * z.shape[0], *z.shape[1:]), z.dtype))
            for z in zero_outs
        ]
        self.out_shapes = [tuple(a.shape) for a in out_avals]

    def run(self, in_maps):
        cat = [
            np.concatenate([np.asarray(m[n]) for m in in_maps], axis=0)
            for n in self.in_names
        ]
        # the accelerator intermittently throws a transient
        # NRT_EXEC_UNIT_UNRECOVERABLE (status 101); retry once
        for attempt in range(3):
            try:
                outs = self.fn(*cat, *self.zeros)
                self.jax.block_until_ready(outs)
                outs = [np.asarray(o) for o in outs]
                break
            except Exception:
                if attempt == 2:
                    raise
                import time as _t
                _t.sleep(5.0)
        return [
            {n: outs[i].reshape(N_CORES, *self.out_shapes[i])[c]
             for i, n in enumerate(self.out_names)}
            for c in range(N_CORES)
        ]


_CACHED_NC = None


def kernel(**inputs) -> np.ndarray:
    global _CACHED, _CACHED_NC
    if _CACHED is None:
        _CACHED_NC = build_nc()
        _CACHED = _Runner(_CACHED_NC)
    in_maps = _prep_inputs(inputs)
    results = _CACHED.run(in_maps)
    # device row (rh, m'') holds output row rho = 8*(m'' % 32) + 4*rh + m''//32
    mpp = np.arange(128)
    rho = np.concatenate([8 * (mpp % 32) + 4 * rh + mpp // 32 for rh in (0, 1)])
    inv = np.argsort(rho)
    out = np.empty((B, S, H * D // 8), np.float32)  # (2, 2048, 128)
    for c in range(N_CORES):
        o = results[c]["outp"]  # [B, 256, 128] in device (rh, m'') row order
        for b in range(B):
            out[b, c::8, :] = o[b][inv]  # rows s3 = 8*rho + c
    return out
